# revision 4
# baseline (speedup 1.0000x reference)
"""Causal self-attention (B=4, T=4096, D=H=1024, fp32) on 8 Trainium2 cores.

Sharding: 2 cores per batch element. Within a batch, the 32 query tiles of
128 rows are interleaved between the 2 cores (core `pair` p takes global
q-tiles p, p+2, p+4, ...), which balances the causal-attention work exactly.
Each core computes the full K/V projection for its batch (replicated between
the 2 cores of a batch), then flash-style attention over its 16 q-tiles.

Numerics: matmuls in bf16 with fp32 PSUM accumulation; softmax without
max-subtraction (scores ~ N(0,1) after the 1/32 scale, exp stays in a safe
fp32 range); exp on ScalarE in fp32, probabilities stored bf16; final
normalization in fp32.  Measured end-to-end error vs the fp32 reference is
~0.5% scale-relative absmax.
"""

import numpy as np

B, T, D, H = 4, 4096, 1024, 1024
P = 128
NCORES = 8


def _emit(ctx, tc, xq, xkv, wq, wk, wv, maskt, ident, outp, T_kv, n_qt):
    import concourse.mybir as mybir

    nc = tc.nc
    f32 = mybir.dt.float32
    bf16 = mybir.dt.bfloat16
    Copy = mybir.ActivationFunctionType.Copy
    Exp = mybir.ActivationFunctionType.Exp
    AX = mybir.AxisListType.X
    SCALE = 1.0 / 32.0  # 1/sqrt(H)

    NKB = T_kv // 128     # kv 128-blocks
    NKC_A = T_kv // 512   # phase-A 512-row projection chunks
    NSC = n_qt // 4       # 512-row query superchunks

    const = ctx.enter_context(tc.tile_pool(name="const", bufs=1))
    persist = ctx.enter_context(tc.tile_pool(name="persist", bufs=1))

    id_sb = const.tile([P, P], bf16, tag="ident")
    nc.sync.dma_start(out=id_sb, in_=ident)
    mask_sb = const.tile([P, 256], bf16, tag="mask")
    nc.sync.dma_start(out=mask_sb, in_=maskt)

    # K^T laid out [h%128, h//128, t]; V laid out [t%128, t//128, h]
    KT = persist.tile([P, 8, T_kv], bf16, tag="KT")
    V = persist.tile([P, NKB, 1024], bf16, tag="V")

    def load_weight(wdram, wsb, stage_pool):
        # DRAM [1024,1024] f32 -> SBUF [128, 8, 1024] bf16 (d = dc*128 + p)
        for g in range(4):
            wf = stage_pool.tile([P, 2, 1024], f32, tag="xf")
            for i in range(2):
                dc = g * 2 + i
                nc.sync.dma_start(out=wf[:, i, :], in_=wdram[dc * P:(dc + 1) * P, :])
                nc.gpsimd.tensor_copy(out=wsb[:, g * 2 + i, :], in_=wf[:, i, :])

    # ---------------- Phase A: K/V projection over all kv rows ----------------
    with tc.tile_pool(name="pa_w", bufs=1) as wpool, \
         tc.tile_pool(name="pa_xf", bufs=2) as xfp, \
         tc.tile_pool(name="pa_xb", bufs=1) as xbp, \
         tc.tile_pool(name="pa_xt", bufs=1) as xtpool, \
         tc.tile_pool(name="pa_pst", bufs=2, space="PSUM") as psA_t, \
         tc.tile_pool(name="pa_psk", bufs=2, space="PSUM") as psA_k, \
         tc.tile_pool(name="pa_psv", bufs=2, space="PSUM") as psA_v:
        wk_sb = wpool.tile([P, 8, 1024], bf16, tag="wk")
        wv_sb = wpool.tile([P, 8, 1024], bf16, tag="wv")
        load_weight(wk, wk_sb, xfp)
        load_weight(wv, wv_sb, xfp)

        for c in range(NKC_A):
            t0 = c * 512
            xt = xtpool.tile([P, 8, 512], bf16, tag="xt")
            for hf in range(2):  # two 256-row halves
                xf = xfp.tile([P, 2, 1024], f32, tag="xf")
                for i in range(2):
                    nc.sync.dma_start(
                        out=xf[:, i, :],
                        in_=xkv[t0 + hf * 256 + i * P: t0 + hf * 256 + (i + 1) * P, :])
                xb = xbp.tile([P, 2, 1024], bf16, tag="xb")
                nc.gpsimd.tensor_copy(out=xb, in_=xf)
                for dc in range(8):
                    tp = psA_t.tile([P, 256], bf16, tag="tp")
                    for i in range(2):
                        nc.tensor.transpose(
                            tp[:, i * P:(i + 1) * P],
                            xb[:, i, dc * P:(dc + 1) * P], id_sb)
                    nc.scalar.activation(
                        out=xt[:, dc, hf * 256:(hf + 1) * 256], in_=tp, func=Copy)
            # K^T_[h, t0:t0+512] = Wk^T @ x^T
            for hc in range(8):
                kp = psA_k.tile([P, 512], f32, tag="kp")
                for dc in range(8):
                    nc.tensor.matmul(
                        kp, lhsT=wk_sb[:, dc, hc * P:(hc + 1) * P],
                        rhs=xt[:, dc, :], start=(dc == 0), stop=(dc == 7))
                nc.vector.tensor_copy(out=KT[:, hc, t0:t0 + 512], in_=kp)
            # V_[t0+i*128, :] = x @ Wv
            for i in range(4):
                vp = psA_v.tile([P, 1024], f32, tag="vp")
                for dc in range(8):
                    for nb in range(2):
                        nc.tensor.matmul(
                            vp[:, nb * 512:(nb + 1) * 512],
                            lhsT=xt[:, dc, i * P:(i + 1) * P],
                            rhs=wv_sb[:, dc, nb * 512:(nb + 1) * 512],
                            start=(dc == 0), stop=(dc == 7))
                nc.vector.tensor_copy(out=V[:, t0 // P + i, :], in_=vp)

    # ---------------- Phase B: Q projection + attention ----------------
    with tc.tile_pool(name="pb_w", bufs=1) as wqp, \
         tc.tile_pool(name="pb_xf", bufs=1) as xfq_p, \
         tc.tile_pool(name="pb_xb", bufs=1) as xbq_p, \
         tc.tile_pool(name="pb_xt", bufs=1) as xtq_p, \
         tc.tile_pool(name="pb_qt", bufs=1) as qt_p, \
         tc.tile_pool(name="pb_p", bufs=3) as pb_p, \
         tc.tile_pool(name="pb_pt", bufs=2) as pt_p, \
         tc.tile_pool(name="pb_sums", bufs=2) as sums_p, \
         tc.tile_pool(name="pb_ob", bufs=2) as ob_p, \
         tc.tile_pool(name="pb_pp", bufs=3, space="PSUM") as ps_pp, \
         tc.tile_pool(name="pb_ps", bufs=3, space="PSUM") as ps_s, \
         tc.tile_pool(name="pb_po", bufs=1, space="PSUM") as ps_o:
        wq_sb = wqp.tile([P, 8, 1024], bf16, tag="wq")
        load_weight(wq, wq_sb, xfq_p)

        for sc in range(NSC):
            # Q^T for this superchunk: [h%128, h//128, 512 local q]
            xtq = xtq_p.tile([P, 8, 512], bf16, tag="xtq")
            for hf in range(2):
                xf = xfq_p.tile([P, 2, 1024], f32, tag="xf")
                for i in range(2):
                    r0 = (sc * 4 + hf * 2 + i) * P
                    nc.sync.dma_start(out=xf[:, i, :], in_=xq[r0:r0 + P, :])
                xb = xbq_p.tile([P, 2, 1024], bf16, tag="xbq")
                nc.gpsimd.tensor_copy(out=xb, in_=xf)
                for dc in range(8):
                    tp = ps_pp.tile([P, 256], bf16, tag="pp")
                    for i in range(2):
                        nc.tensor.transpose(
                            tp[:, i * P:(i + 1) * P],
                            xb[:, i, dc * P:(dc + 1) * P], id_sb)
                    nc.scalar.activation(
                        out=xtq[:, dc, hf * 256:(hf + 1) * 256], in_=tp, func=Copy)
            qt = qt_p.tile([P, 8, 512], bf16, tag="qt")
            for hc in range(8):
                qp = ps_pp.tile([P, 512], f32, tag="pp")
                for dc in range(8):
                    nc.tensor.matmul(
                        qp, lhsT=wq_sb[:, dc, hc * P:(hc + 1) * P],
                        rhs=xtq[:, dc, :], start=(dc == 0), stop=(dc == 7))
                nc.vector.tensor_copy(out=qt[:, hc, :], in_=qp)

            for o in range(4):
                j = sc * 4 + o
                nch = j + 1
                sums = sums_p.tile([P, 16], f32, tag="sums")
                op = ps_o.tile([P, 1024], f32, tag="op")

                def s_mm(c):
                    sp = ps_s.tile([P, 256], f32, tag="sp")
                    for hc in range(8):
                        nc.tensor.matmul(
                            sp, lhsT=qt[:, hc, o * P:(o + 1) * P],
                            rhs=KT[:, hc, c * 256:(c + 1) * 256],
                            start=(hc == 0), stop=(hc == 7))
                    return sp

                def softmax(c, sp):
                    pb = pb_p.tile([P, 256], bf16, tag="pb")
                    if c < nch - 1:
                        nc.scalar.activation(out=pb, in_=sp, func=Exp,
                                             scale=SCALE, accum_out=sums[:, c:c + 1])
                    else:
                        nc.scalar.activation(out=pb, in_=sp, func=Exp, scale=SCALE)
                        nc.vector.tensor_mul(pb, pb, mask_sb)
                        nc.vector.reduce_sum(out=sums[:, c:c + 1], in_=pb, axis=AX)
                    return pb

                def pv(c, pb):
                    ptp = ps_pp.tile([P, 256], bf16, tag="pp")
                    nc.tensor.transpose(ptp[:, 0:P], pb[:, 0:P], id_sb)
                    nc.tensor.transpose(ptp[:, P:256], pb[:, P:256], id_sb)
                    pt = pt_p.tile([P, 256], bf16, tag="pt")
                    nc.vector.tensor_copy(out=pt, in_=ptp)
                    for kl in range(2):
                        kb = c * 2 + kl
                        for nb in range(2):
                            nc.tensor.matmul(
                                op[:, nb * 512:(nb + 1) * 512],
                                lhsT=pt[:, kl * P:(kl + 1) * P],
                                rhs=V[:, kb, nb * 512:(nb + 1) * 512],
                                start=(c == 0 and kl == 0),
                                stop=(c == nch - 1 and kl == 1))

                sps = {0: s_mm(0)}
                pbs = {0: softmax(0, sps[0])}
                if nch > 1:
                    sps[1] = s_mm(1)
                    pbs[1] = softmax(1, sps[1])
                for c in range(nch):
                    pv(c, pbs[c])
                    if c + 2 < nch:
                        sps[c + 2] = s_mm(c + 2)
                        pbs[c + 2] = softmax(c + 2, sps[c + 2])

                tot = sums_p.tile([P, 1], f32, tag="tot")
                nc.vector.reduce_sum(out=tot, in_=sums[:, 0:nch], axis=AX)
                rec = sums_p.tile([P, 1], f32, tag="rec")
                nc.vector.reciprocal(out=rec, in_=tot)
                ob = ob_p.tile([P, 1024], f32, tag="ob")
                nc.scalar.activation(out=ob, in_=op, func=Copy, scale=rec)
                nc.sync.dma_start(out=outp[j * P:(j + 1) * P, :], in_=ob)


def build_module(T_kv=T, n_qt=None):
    from contextlib import ExitStack
    import concourse.tile as tile
    import concourse.mybir as mybir
    from concourse import bacc

    if n_qt is None:
        n_qt = T_kv // 256
    dt = mybir.dt
    nc = bacc.Bacc("TRN2", target_bir_lowering=False, debug=False,
                   num_devices=NCORES)
    xq = nc.dram_tensor("xq", [n_qt * P, D], dt.float32, kind="ExternalInput").ap()
    xkv = nc.dram_tensor("xkv", [T_kv, D], dt.float32, kind="ExternalInput").ap()
    wq = nc.dram_tensor("wq", [D, H], dt.float32, kind="ExternalInput").ap()
    wk = nc.dram_tensor("wk", [D, H], dt.float32, kind="ExternalInput").ap()
    wv = nc.dram_tensor("wv", [D, H], dt.float32, kind="ExternalInput").ap()
    maskt = nc.dram_tensor("maskt", [P, 256], dt.bfloat16, kind="ExternalInput").ap()
    ident = nc.dram_tensor("ident", [P, P], dt.bfloat16, kind="ExternalInput").ap()
    outp = nc.dram_tensor("outp", [n_qt * P, H], dt.float32, kind="ExternalOutput").ap()

    with tile.TileContext(nc) as tc:
        with ExitStack() as ctx:
            _emit(ctx, tc, xq, xkv, wq, wk, wv, maskt, ident, outp, T_kv, n_qt)
    nc.compile()
    return nc


def host_inputs(x, Wq, Wk, Wv, T_kv=T, n_qt=None, n_batch=None):
    """Build the per-core input maps for run_bass_kernel_spmd."""
    import ml_dtypes
    bf = ml_dtypes.bfloat16
    if n_qt is None:
        n_qt = T_kv // 256
    if n_batch is None:
        n_batch = x.shape[0]
    eye = np.eye(P, dtype=np.float32).astype(bf)
    tril = np.tril(np.ones((P, P), np.float32))
    m = [np.concatenate([tril, np.zeros((P, P), np.float32)], 1).astype(bf),
         np.concatenate([np.ones((P, P), np.float32), tril], 1).astype(bf)]
    in_maps = []
    for c in range(NCORES):
        b, pair = (c // 2) % n_batch, c % 2
        qrows = np.concatenate(
            [x[b, (2 * j + pair) * P:(2 * j + pair + 1) * P, :] for j in range(n_qt)], 0)
        in_maps.append({
            "xq": np.ascontiguousarray(qrows),
            "xkv": np.ascontiguousarray(x[b]),
            "wq": Wq, "wk": Wk, "wv": Wv,
            "maskt": m[pair], "ident": eye,
        })
    return in_maps


def gather_output(results, T_kv=T, n_qt=None, n_batch=B):
    if n_qt is None:
        n_qt = T_kv // 256
    out = np.empty((n_batch, T_kv, H), np.float32)
    for c in range(2 * n_batch):
        b, pair = c // 2, c % 2
        r = results[c]["outp"]
        for j in range(n_qt):
            out[b, (2 * j + pair) * P:(2 * j + pair + 1) * P, :] = \
                r[j * P:(j + 1) * P, :]
    return out


_NC_CACHE = {}


def kernel(x, Wq, Wk, Wv):
    from concourse.bass_utils import run_bass_kernel_spmd

    x = np.asarray(x, dtype=np.float32)
    Wq = np.asarray(Wq, dtype=np.float32)
    Wk = np.asarray(Wk, dtype=np.float32)
    Wv = np.asarray(Wv, dtype=np.float32)

    if "nc" not in _NC_CACHE:
        _NC_CACHE["nc"] = build_module()
    nc = _NC_CACHE["nc"]

    in_maps = host_inputs(x, Wq, Wk, Wv)
    res = run_bass_kernel_spmd(nc, in_maps, core_ids=list(range(NCORES)))
    return gather_output(res.results)


# revision 10
# speedup vs baseline: 116.7277x; 116.7277x over previous
"""Causal self-attention (B=4, T=4096, D=H=1024, fp32) on 8 Trainium2 cores.

Sharding: 2 cores per batch element. Within a batch, the 32 query tiles of
128 rows are interleaved between the 2 cores (core `pair` p takes global
q-tiles p, p+2, p+4, ...), which balances the causal-attention work exactly.
Each core computes the full K/V projection for its batch (replicated between
the 2 cores of a batch), then flash-style attention over its 16 q-tiles.

Numerics: matmuls in bf16 with fp32 PSUM accumulation; softmax without
max-subtraction (scores ~ N(0,1) after the 1/32 scale, exp stays in a safe
fp32 range); exp on ScalarE in fp32, probabilities stored bf16; final
normalization in fp32.  Measured end-to-end error vs the fp32 reference is
~0.5% scale-relative absmax.
"""

import numpy as np

B, T, D, H = 4, 4096, 1024, 1024
P = 128
NCORES = 8


DEFAULT_CFG = dict(
    phases="AB",
    pa_xf_bufs=2, pa_xb_bufs=1, pa_xt_bufs=1,
    pa_pst_bufs=2, pa_psk_bufs=2, pa_psv_bufs=2,
    pb_p_bufs=3, pb_pt_bufs=2, pb_ob_bufs=2,
    pb_pp_bufs=3, pb_ps_bufs=3, pb_po_bufs=1,
    s_ahead=2,
)


def _emit(ctx, tc, xq, xkv, wq, wk, wv, maskt, ident, outp, T_kv, n_qt, cfg):
    import concourse.mybir as mybir

    nc = tc.nc
    f32 = mybir.dt.float32
    bf16 = mybir.dt.bfloat16
    Copy = mybir.ActivationFunctionType.Copy
    Exp = mybir.ActivationFunctionType.Exp
    AX = mybir.AxisListType.X
    SCALE = 1.0 / 32.0  # 1/sqrt(H)

    NKB = T_kv // 128     # kv 128-blocks
    NKC_A = T_kv // 512   # phase-A 512-row projection chunks
    NSC = n_qt // 4       # 512-row query superchunks
    if "A" not in cfg["phases"]:
        NKC_A = 0
    if "B" not in cfg["phases"]:
        NSC = 0

    const = ctx.enter_context(tc.tile_pool(name="const", bufs=1))
    persist = ctx.enter_context(tc.tile_pool(name="persist", bufs=1))

    id_sb = const.tile([P, P], bf16, tag="ident")
    nc.sync.dma_start(out=id_sb, in_=ident)
    mask_sb = const.tile([P, 256], bf16, tag="mask")
    nc.sync.dma_start(out=mask_sb, in_=maskt)

    # K^T laid out [h%128, h//128, t]; V laid out [t%128, t//128, h]
    KT = persist.tile([P, 8, T_kv], bf16, tag="KT")
    V = persist.tile([P, NKB, 1024], bf16, tag="V")

    def load_weight(wdram, wsb, stage_pool):
        # DRAM [1024,1024] f32 -> SBUF [128, 8, 1024] bf16 (d = dc*128 + p)
        for g in range(4):
            wf = stage_pool.tile([P, 2, 1024], f32, tag="xf")
            for i in range(2):
                dc = g * 2 + i
                nc.sync.dma_start(out=wf[:, i, :], in_=wdram[dc * P:(dc + 1) * P, :])
                nc.gpsimd.tensor_copy(out=wsb[:, g * 2 + i, :], in_=wf[:, i, :])

    # ---------------- Phase A: K/V projection over all kv rows ----------------
    with tc.tile_pool(name="pa_w", bufs=1) as wpool, \
         tc.tile_pool(name="pa_xf", bufs=cfg["pa_xf_bufs"]) as xfp, \
         tc.tile_pool(name="pa_xb", bufs=cfg["pa_xb_bufs"]) as xbp, \
         tc.tile_pool(name="pa_xt", bufs=cfg["pa_xt_bufs"]) as xtpool, \
         tc.tile_pool(name="pa_pst", bufs=cfg["pa_pst_bufs"], space="PSUM") as psA_t, \
         tc.tile_pool(name="pa_psk", bufs=cfg["pa_psk_bufs"], space="PSUM") as psA_k, \
         tc.tile_pool(name="pa_psv", bufs=cfg["pa_psv_bufs"], space="PSUM") as psA_v:
        wk_sb = wpool.tile([P, 8, 1024], bf16, tag="wk")
        wv_sb = wpool.tile([P, 8, 1024], bf16, tag="wv")
        load_weight(wk, wk_sb, xfp)
        load_weight(wv, wv_sb, xfp)

        for c in range(NKC_A):
            t0 = c * 512
            xt = xtpool.tile([P, 8, 512], bf16, tag="xt")
            for hf in range(2):  # two 256-row halves
                xf = xfp.tile([P, 2, 1024], f32, tag="xf")
                for i in range(2):
                    nc.sync.dma_start(
                        out=xf[:, i, :],
                        in_=xkv[t0 + hf * 256 + i * P: t0 + hf * 256 + (i + 1) * P, :])
                xb = xbp.tile([P, 2, 1024], bf16, tag="xb")
                nc.gpsimd.tensor_copy(out=xb, in_=xf)
                for dc in range(8):
                    tp = psA_t.tile([P, 256], bf16, tag="tp")
                    for i in range(2):
                        nc.tensor.transpose(
                            tp[:, i * P:(i + 1) * P],
                            xb[:, i, dc * P:(dc + 1) * P], id_sb)
                    nc.scalar.activation(
                        out=xt[:, dc, hf * 256:(hf + 1) * 256], in_=tp, func=Copy)
            # K^T_[h, t0:t0+512] = Wk^T @ x^T
            for hc in range(8):
                kp = psA_k.tile([P, 512], f32, tag="kp")
                for dc in range(8):
                    nc.tensor.matmul(
                        kp, lhsT=wk_sb[:, dc, hc * P:(hc + 1) * P],
                        rhs=xt[:, dc, :], start=(dc == 0), stop=(dc == 7))
                nc.vector.tensor_copy(out=KT[:, hc, t0:t0 + 512], in_=kp)
            # V_[t0+i*128, :] = x @ Wv
            for i in range(4):
                vp = psA_v.tile([P, 1024], f32, tag="vp")
                for dc in range(8):
                    for nb in range(2):
                        nc.tensor.matmul(
                            vp[:, nb * 512:(nb + 1) * 512],
                            lhsT=xt[:, dc, i * P:(i + 1) * P],
                            rhs=wv_sb[:, dc, nb * 512:(nb + 1) * 512],
                            start=(dc == 0), stop=(dc == 7))
                nc.vector.tensor_copy(out=V[:, t0 // P + i, :], in_=vp)

    # ---------------- Phase B: Q projection + attention ----------------
    with tc.tile_pool(name="pb_w", bufs=1) as wqp, \
         tc.tile_pool(name="pb_xf", bufs=1) as xfq_p, \
         tc.tile_pool(name="pb_xb", bufs=1) as xbq_p, \
         tc.tile_pool(name="pb_xt", bufs=1) as xtq_p, \
         tc.tile_pool(name="pb_qt", bufs=1) as qt_p, \
         tc.tile_pool(name="pb_p", bufs=cfg["pb_p_bufs"]) as pb_p, \
         tc.tile_pool(name="pb_pt", bufs=cfg["pb_pt_bufs"]) as pt_p, \
         tc.tile_pool(name="pb_sums", bufs=2) as sums_p, \
         tc.tile_pool(name="pb_ob", bufs=cfg["pb_ob_bufs"]) as ob_p, \
         tc.tile_pool(name="pb_pp", bufs=cfg["pb_pp_bufs"], space="PSUM") as ps_pp, \
         tc.tile_pool(name="pb_ps", bufs=cfg["pb_ps_bufs"], space="PSUM") as ps_s, \
         tc.tile_pool(name="pb_po", bufs=cfg["pb_po_bufs"], space="PSUM") as ps_o:
        wq_sb = wqp.tile([P, 8, 1024], bf16, tag="wq")
        load_weight(wq, wq_sb, xfq_p)

        for sc in range(NSC):
            # Q^T for this superchunk: [h%128, h//128, 512 local q]
            xtq = xtq_p.tile([P, 8, 512], bf16, tag="xtq")
            for hf in range(2):
                xf = xfq_p.tile([P, 2, 1024], f32, tag="xf")
                for i in range(2):
                    r0 = (sc * 4 + hf * 2 + i) * P
                    nc.sync.dma_start(out=xf[:, i, :], in_=xq[r0:r0 + P, :])
                xb = xbq_p.tile([P, 2, 1024], bf16, tag="xbq")
                nc.gpsimd.tensor_copy(out=xb, in_=xf)
                for dc in range(8):
                    tp = ps_pp.tile([P, 256], bf16, tag="pp")
                    for i in range(2):
                        nc.tensor.transpose(
                            tp[:, i * P:(i + 1) * P],
                            xb[:, i, dc * P:(dc + 1) * P], id_sb)
                    nc.scalar.activation(
                        out=xtq[:, dc, hf * 256:(hf + 1) * 256], in_=tp, func=Copy)
            qt = qt_p.tile([P, 8, 512], bf16, tag="qt")
            for hc in range(8):
                qp = ps_pp.tile([P, 512], f32, tag="pp")
                for dc in range(8):
                    nc.tensor.matmul(
                        qp, lhsT=wq_sb[:, dc, hc * P:(hc + 1) * P],
                        rhs=xtq[:, dc, :], start=(dc == 0), stop=(dc == 7))
                nc.vector.tensor_copy(out=qt[:, hc, :], in_=qp)

            for o in range(4):
                j = sc * 4 + o
                nch = j + 1
                sums = sums_p.tile([P, 16], f32, tag="sums")
                op = ps_o.tile([P, 1024], f32, tag="op")

                def s_mm(c):
                    sp = ps_s.tile([P, 256], f32, tag="sp")
                    for hc in range(8):
                        nc.tensor.matmul(
                            sp, lhsT=qt[:, hc, o * P:(o + 1) * P],
                            rhs=KT[:, hc, c * 256:(c + 1) * 256],
                            start=(hc == 0), stop=(hc == 7))
                    return sp

                def softmax(c, sp):
                    pb = pb_p.tile([P, 256], bf16, tag="pb")
                    if c < nch - 1:
                        nc.scalar.activation(out=pb, in_=sp, func=Exp,
                                             scale=SCALE, accum_out=sums[:, c:c + 1])
                    else:
                        nc.scalar.activation(out=pb, in_=sp, func=Exp, scale=SCALE)
                        nc.vector.tensor_mul(pb, pb, mask_sb)
                        nc.vector.reduce_sum(out=sums[:, c:c + 1], in_=pb, axis=AX)
                    return pb

                def pv(c, pb):
                    ptp = ps_pp.tile([P, 256], bf16, tag="pp")
                    nc.tensor.transpose(ptp[:, 0:P], pb[:, 0:P], id_sb)
                    nc.tensor.transpose(ptp[:, P:256], pb[:, P:256], id_sb)
                    pt = pt_p.tile([P, 256], bf16, tag="pt")
                    nc.vector.tensor_copy(out=pt, in_=ptp)
                    for kl in range(2):
                        kb = c * 2 + kl
                        for nb in range(2):
                            nc.tensor.matmul(
                                op[:, nb * 512:(nb + 1) * 512],
                                lhsT=pt[:, kl * P:(kl + 1) * P],
                                rhs=V[:, kb, nb * 512:(nb + 1) * 512],
                                start=(c == 0 and kl == 0),
                                stop=(c == nch - 1 and kl == 1))

                ahead = cfg["s_ahead"]
                sps, pbs = {}, {}
                for c in range(min(ahead, nch)):
                    sps[c] = s_mm(c)
                    pbs[c] = softmax(c, sps[c])
                for c in range(nch):
                    pv(c, pbs[c])
                    if c + ahead < nch:
                        sps[c + ahead] = s_mm(c + ahead)
                        pbs[c + ahead] = softmax(c + ahead, sps[c + ahead])

                tot = sums_p.tile([P, 1], f32, tag="tot")
                nc.vector.reduce_sum(out=tot, in_=sums[:, 0:nch], axis=AX)
                rec = sums_p.tile([P, 1], f32, tag="rec")
                nc.vector.reciprocal(out=rec, in_=tot)
                ob = ob_p.tile([P, 1024], f32, tag="ob")
                nc.scalar.activation(out=ob, in_=op, func=Copy, scale=rec)
                nc.sync.dma_start(out=outp[j * P:(j + 1) * P, :], in_=ob)


def build_module(T_kv=T, n_qt=None, cfg=None):
    from contextlib import ExitStack
    import concourse.tile as tile
    import concourse.mybir as mybir
    from concourse import bacc

    if n_qt is None:
        n_qt = T_kv // 256
    full_cfg = dict(DEFAULT_CFG)
    if cfg:
        full_cfg.update(cfg)
    cfg = full_cfg
    dt = mybir.dt
    nc = bacc.Bacc("TRN2", target_bir_lowering=False, debug=False,
                   num_devices=NCORES)
    xq = nc.dram_tensor("xq", [n_qt * P, D], dt.float32, kind="ExternalInput").ap()
    xkv = nc.dram_tensor("xkv", [T_kv, D], dt.float32, kind="ExternalInput").ap()
    wq = nc.dram_tensor("wq", [D, H], dt.float32, kind="ExternalInput").ap()
    wk = nc.dram_tensor("wk", [D, H], dt.float32, kind="ExternalInput").ap()
    wv = nc.dram_tensor("wv", [D, H], dt.float32, kind="ExternalInput").ap()
    maskt = nc.dram_tensor("maskt", [P, 256], dt.bfloat16, kind="ExternalInput").ap()
    ident = nc.dram_tensor("ident", [P, P], dt.bfloat16, kind="ExternalInput").ap()
    outp = nc.dram_tensor("outp", [n_qt * P, H], dt.float32, kind="ExternalOutput").ap()

    with tile.TileContext(nc) as tc:
        with ExitStack() as ctx:
            _emit(ctx, tc, xq, xkv, wq, wk, wv, maskt, ident, outp, T_kv, n_qt,
                  cfg)
    nc.compile()
    return nc


def host_inputs(x, Wq, Wk, Wv, T_kv=T, n_qt=None, n_batch=None):
    """Build the per-core input maps for run_bass_kernel_spmd."""
    import ml_dtypes
    bf = ml_dtypes.bfloat16
    if n_qt is None:
        n_qt = T_kv // 256
    if n_batch is None:
        n_batch = x.shape[0]
    eye = np.eye(P, dtype=np.float32).astype(bf)
    tril = np.tril(np.ones((P, P), np.float32))
    m = [np.concatenate([tril, np.zeros((P, P), np.float32)], 1).astype(bf),
         np.concatenate([np.ones((P, P), np.float32), tril], 1).astype(bf)]
    in_maps = []
    for c in range(NCORES):
        b, pair = (c // 2) % n_batch, c % 2
        qrows = np.concatenate(
            [x[b, (2 * j + pair) * P:(2 * j + pair + 1) * P, :] for j in range(n_qt)], 0)
        in_maps.append({
            "xq": np.ascontiguousarray(qrows),
            "xkv": np.ascontiguousarray(x[b]),
            "wq": Wq, "wk": Wk, "wv": Wv,
            "maskt": m[pair], "ident": eye,
        })
    return in_maps


def gather_output(results, T_kv=T, n_qt=None, n_batch=B):
    if n_qt is None:
        n_qt = T_kv // 256
    out = np.empty((n_batch, T_kv, H), np.float32)
    for c in range(2 * n_batch):
        b, pair = c // 2, c % 2
        r = results[c]["outp"]
        for j in range(n_qt):
            out[b, (2 * j + pair) * P:(2 * j + pair + 1) * P, :] = \
                r[j * P:(j + 1) * P, :]
    return out


_NC_CACHE = {}


def kernel(x, Wq, Wk, Wv):
    from concourse.bass_utils import run_bass_kernel_spmd

    x = np.asarray(x, dtype=np.float32)
    Wq = np.asarray(Wq, dtype=np.float32)
    Wk = np.asarray(Wk, dtype=np.float32)
    Wv = np.asarray(Wv, dtype=np.float32)

    if "nc" not in _NC_CACHE:
        _NC_CACHE["nc"] = build_module()
    nc = _NC_CACHE["nc"]

    in_maps = host_inputs(x, Wq, Wk, Wv)
    res = run_bass_kernel_spmd(nc, in_maps, core_ids=list(range(NCORES)))
    return gather_output(res.results)


# revision 11
# speedup vs baseline: 150.8378x; 1.2922x over previous
"""Causal self-attention (B=4, T=4096, D=H=1024, fp32) on 8 Trainium2 cores.

Sharding: 2 cores per batch element. Within a batch, the 32 query tiles of
128 rows are interleaved between the 2 cores (core `pair` p takes global
q-tiles p, p+2, p+4, ...), which balances the causal-attention work exactly.
Each core computes the full K/V projection for its batch (replicated between
the 2 cores of a batch), then flash-style attention over its 16 q-tiles.

Numerics: x and the weights are cast to bf16 on the host; all matmuls run
bf16 with fp32 PSUM accumulation. Softmax skips max-subtraction (scores are
~N(0,1) after the 1/32 scale so exp stays in a safe fp32 range); exp runs on
ScalarE in fp32, probabilities are stored bf16, and the final normalization
is fp32. Measured error vs the fp32 reference: ~0.4% scale-relative absmax.
"""

import numpy as np

B, T, D, H = 4, 4096, 1024, 1024
P = 128
NCORES = 8


DEFAULT_CFG = dict(
    phases="AB",
    pa_xb_bufs=8, pa_xt_bufs=1,
    pa_pst_bufs=2, pa_psk_bufs=2, pa_psv_bufs=2,
    pb_xb_bufs=8, pb_xt_bufs=1, pb_qt_bufs=1,
    pb_p_bufs=3, pb_pt_bufs=2, pb_ob_bufs=2,
    pb_pp_bufs=3, pb_ps_bufs=3, pb_po_bufs=1,
    s_ahead=2,
)


def _emit(ctx, tc, xq, xkv, wq, wk, wv, maskt, ident, outp, T_kv, n_qt, cfg):
    import concourse.mybir as mybir

    nc = tc.nc
    f32 = mybir.dt.float32
    bf16 = mybir.dt.bfloat16
    Copy = mybir.ActivationFunctionType.Copy
    Exp = mybir.ActivationFunctionType.Exp
    AX = mybir.AxisListType.X
    SCALE = 1.0 / 32.0  # 1/sqrt(H)

    NKB = T_kv // 128     # kv 128-blocks
    NKC_A = T_kv // 512   # phase-A 512-row projection chunks
    NSC = n_qt // 4       # 512-row query superchunks
    if "A" not in cfg["phases"]:
        NKC_A = 0
    if "B" not in cfg["phases"]:
        NSC = 0

    const = ctx.enter_context(tc.tile_pool(name="const", bufs=1))
    persist = ctx.enter_context(tc.tile_pool(name="persist", bufs=1))

    id_sb = const.tile([P, P], bf16, tag="ident")
    nc.sync.dma_start(out=id_sb, in_=ident)
    mask_sb = const.tile([P, 256], bf16, tag="mask")
    nc.sync.dma_start(out=mask_sb, in_=maskt)

    # K^T laid out [h%128, h//128, t]; V laid out [t%128, t//128, h]
    KT = persist.tile([P, 8, T_kv], bf16, tag="KT")
    V = persist.tile([P, NKB, 1024], bf16, tag="V")

    def load_weight(wdram, wsb):
        # DRAM [1024,1024] bf16 -> SBUF [128, 8, 1024] (d = dc*128 + p)
        for dc in range(8):
            nc.sync.dma_start(out=wsb[:, dc, :], in_=wdram[dc * P:(dc + 1) * P, :])

    # ---------------- Phase A: K/V projection over all kv rows ----------------
    with tc.tile_pool(name="pa_w", bufs=1) as wpool, \
         tc.tile_pool(name="pa_xb", bufs=cfg["pa_xb_bufs"]) as xbp, \
         tc.tile_pool(name="pa_xt", bufs=cfg["pa_xt_bufs"]) as xtpool, \
         tc.tile_pool(name="pa_pst", bufs=cfg["pa_pst_bufs"], space="PSUM") as psA_t, \
         tc.tile_pool(name="pa_psk", bufs=cfg["pa_psk_bufs"], space="PSUM") as psA_k, \
         tc.tile_pool(name="pa_psv", bufs=cfg["pa_psv_bufs"], space="PSUM") as psA_v:
        wk_sb = wpool.tile([P, 8, 1024], bf16, tag="wk")
        wv_sb = wpool.tile([P, 8, 1024], bf16, tag="wv")
        load_weight(wk, wk_sb)
        load_weight(wv, wv_sb)

        for c in range(NKC_A):
            t0 = c * 512
            xt = xtpool.tile([P, 8, 512], bf16, tag="xt")
            xbs = []
            for i in range(4):
                xb = xbp.tile([P, 1024], bf16, tag="xb")
                nc.sync.dma_start(out=xb, in_=xkv[t0 + i * P: t0 + (i + 1) * P, :])
                xbs.append(xb)
            for hf in range(2):
                for dc in range(8):
                    tp = psA_t.tile([P, 256], bf16, tag="tp")
                    for i in range(2):
                        nc.tensor.transpose(
                            tp[:, i * P:(i + 1) * P],
                            xbs[hf * 2 + i][:, dc * P:(dc + 1) * P], id_sb)
                    nc.scalar.activation(
                        out=xt[:, dc, hf * 256:(hf + 1) * 256], in_=tp, func=Copy)
            # K^T_[h, t0:t0+512] = Wk^T @ x^T
            for hc in range(8):
                kp = psA_k.tile([P, 512], f32, tag="kp")
                for dc in range(8):
                    nc.tensor.matmul(
                        kp, lhsT=wk_sb[:, dc, hc * P:(hc + 1) * P],
                        rhs=xt[:, dc, :], start=(dc == 0), stop=(dc == 7))
                nc.vector.tensor_copy(out=KT[:, hc, t0:t0 + 512], in_=kp)
            # V_[t0+i*128, :] = x @ Wv
            for i in range(4):
                vp = psA_v.tile([P, 1024], f32, tag="vp")
                for dc in range(8):
                    for nb in range(2):
                        nc.tensor.matmul(
                            vp[:, nb * 512:(nb + 1) * 512],
                            lhsT=xt[:, dc, i * P:(i + 1) * P],
                            rhs=wv_sb[:, dc, nb * 512:(nb + 1) * 512],
                            start=(dc == 0), stop=(dc == 7))
                nc.vector.tensor_copy(out=V[:, t0 // P + i, :], in_=vp)

    # ---------------- Phase B: Q projection + attention ----------------
    with tc.tile_pool(name="pb_w", bufs=1) as wqp, \
         tc.tile_pool(name="pb_xb", bufs=cfg["pb_xb_bufs"]) as xbq_p, \
         tc.tile_pool(name="pb_xt", bufs=cfg["pb_xt_bufs"]) as xtq_p, \
         tc.tile_pool(name="pb_qt", bufs=cfg["pb_qt_bufs"]) as qt_p, \
         tc.tile_pool(name="pb_p", bufs=cfg["pb_p_bufs"]) as pb_p, \
         tc.tile_pool(name="pb_pt", bufs=cfg["pb_pt_bufs"]) as pt_p, \
         tc.tile_pool(name="pb_sums", bufs=2) as sums_p, \
         tc.tile_pool(name="pb_ob", bufs=cfg["pb_ob_bufs"]) as ob_p, \
         tc.tile_pool(name="pb_pp", bufs=cfg["pb_pp_bufs"], space="PSUM") as ps_pp, \
         tc.tile_pool(name="pb_ps", bufs=cfg["pb_ps_bufs"], space="PSUM") as ps_s, \
         tc.tile_pool(name="pb_po", bufs=cfg["pb_po_bufs"], space="PSUM") as ps_o:
        wq_sb = wqp.tile([P, 8, 1024], bf16, tag="wq")
        load_weight(wq, wq_sb)

        for sc in range(NSC):
            # Q^T for this superchunk: [h%128, h//128, 512 local q]
            xtq = xtq_p.tile([P, 8, 512], bf16, tag="xtq")
            xbqs = []
            for i in range(4):
                xb = xbq_p.tile([P, 1024], bf16, tag="xbq")
                r0 = (sc * 4 + i) * P
                nc.sync.dma_start(out=xb, in_=xq[r0:r0 + P, :])
                xbqs.append(xb)
            for hf in range(2):
                for dc in range(8):
                    tp = ps_pp.tile([P, 256], bf16, tag="pp")
                    for i in range(2):
                        nc.tensor.transpose(
                            tp[:, i * P:(i + 1) * P],
                            xbqs[hf * 2 + i][:, dc * P:(dc + 1) * P], id_sb)
                    nc.scalar.activation(
                        out=xtq[:, dc, hf * 256:(hf + 1) * 256], in_=tp, func=Copy)
            qt = qt_p.tile([P, 8, 512], bf16, tag="qt")
            for hc in range(8):
                qp = ps_pp.tile([P, 512], f32, tag="pp")
                for dc in range(8):
                    nc.tensor.matmul(
                        qp, lhsT=wq_sb[:, dc, hc * P:(hc + 1) * P],
                        rhs=xtq[:, dc, :], start=(dc == 0), stop=(dc == 7))
                nc.vector.tensor_copy(out=qt[:, hc, :], in_=qp)

            for o in range(4):
                j = sc * 4 + o
                nch = j + 1
                sums = sums_p.tile([P, 16], f32, tag="sums")
                op = ps_o.tile([P, 1024], f32, tag="op")

                def s_mm(c):
                    sp = ps_s.tile([P, 256], f32, tag="sp")
                    for hc in range(8):
                        nc.tensor.matmul(
                            sp, lhsT=qt[:, hc, o * P:(o + 1) * P],
                            rhs=KT[:, hc, c * 256:(c + 1) * 256],
                            start=(hc == 0), stop=(hc == 7))
                    return sp

                def softmax(c, sp):
                    pb = pb_p.tile([P, 256], bf16, tag="pb")
                    if c < nch - 1:
                        nc.scalar.activation(out=pb, in_=sp, func=Exp,
                                             scale=SCALE, accum_out=sums[:, c:c + 1])
                    else:
                        nc.scalar.activation(out=pb, in_=sp, func=Exp, scale=SCALE)
                        nc.vector.tensor_mul(pb, pb, mask_sb)
                        nc.vector.reduce_sum(out=sums[:, c:c + 1], in_=pb, axis=AX)
                    return pb

                def pv(c, pb):
                    ptp = ps_pp.tile([P, 256], bf16, tag="pp")
                    nc.tensor.transpose(ptp[:, 0:P], pb[:, 0:P], id_sb)
                    nc.tensor.transpose(ptp[:, P:256], pb[:, P:256], id_sb)
                    pt = pt_p.tile([P, 256], bf16, tag="pt")
                    nc.vector.tensor_copy(out=pt, in_=ptp)
                    for kl in range(2):
                        kb = c * 2 + kl
                        for nb in range(2):
                            nc.tensor.matmul(
                                op[:, nb * 512:(nb + 1) * 512],
                                lhsT=pt[:, kl * P:(kl + 1) * P],
                                rhs=V[:, kb, nb * 512:(nb + 1) * 512],
                                start=(c == 0 and kl == 0),
                                stop=(c == nch - 1 and kl == 1))

                ahead = cfg["s_ahead"]
                sps, pbs = {}, {}
                for c in range(min(ahead, nch)):
                    sps[c] = s_mm(c)
                    pbs[c] = softmax(c, sps[c])
                for c in range(nch):
                    pv(c, pbs[c])
                    if c + ahead < nch:
                        sps[c + ahead] = s_mm(c + ahead)
                        pbs[c + ahead] = softmax(c + ahead, sps[c + ahead])

                tot = sums_p.tile([P, 1], f32, tag="tot")
                nc.vector.reduce_sum(out=tot, in_=sums[:, 0:nch], axis=AX)
                rec = sums_p.tile([P, 1], f32, tag="rec")
                nc.vector.reciprocal(out=rec, in_=tot)
                ob = ob_p.tile([P, 1024], f32, tag="ob")
                nc.scalar.activation(out=ob, in_=op, func=Copy, scale=rec)
                nc.sync.dma_start(out=outp[j * P:(j + 1) * P, :], in_=ob)


def build_module(T_kv=T, n_qt=None, cfg=None):
    from contextlib import ExitStack
    import concourse.tile as tile
    import concourse.mybir as mybir
    from concourse import bacc

    if n_qt is None:
        n_qt = T_kv // 256
    full_cfg = dict(DEFAULT_CFG)
    if cfg:
        full_cfg.update(cfg)
    cfg = full_cfg
    dt = mybir.dt
    nc = bacc.Bacc("TRN2", target_bir_lowering=False, debug=False,
                   num_devices=NCORES)
    xq = nc.dram_tensor("xq", [n_qt * P, D], dt.bfloat16, kind="ExternalInput").ap()
    xkv = nc.dram_tensor("xkv", [T_kv, D], dt.bfloat16, kind="ExternalInput").ap()
    wq = nc.dram_tensor("wq", [D, H], dt.bfloat16, kind="ExternalInput").ap()
    wk = nc.dram_tensor("wk", [D, H], dt.bfloat16, kind="ExternalInput").ap()
    wv = nc.dram_tensor("wv", [D, H], dt.bfloat16, kind="ExternalInput").ap()
    maskt = nc.dram_tensor("maskt", [P, 256], dt.bfloat16, kind="ExternalInput").ap()
    ident = nc.dram_tensor("ident", [P, P], dt.bfloat16, kind="ExternalInput").ap()
    outp = nc.dram_tensor("outp", [n_qt * P, H], dt.float32, kind="ExternalOutput").ap()

    with tile.TileContext(nc) as tc:
        with ExitStack() as ctx:
            _emit(ctx, tc, xq, xkv, wq, wk, wv, maskt, ident, outp, T_kv, n_qt,
                  cfg)
    nc.compile()
    return nc


def host_inputs(x, Wq, Wk, Wv, T_kv=T, n_qt=None, n_batch=None):
    """Build the per-core input maps for run_bass_kernel_spmd."""
    import ml_dtypes
    bf = ml_dtypes.bfloat16
    if n_qt is None:
        n_qt = T_kv // 256
    if n_batch is None:
        n_batch = x.shape[0]
    eye = np.eye(P, dtype=np.float32).astype(bf)
    tril = np.tril(np.ones((P, P), np.float32))
    m = [np.concatenate([tril, np.zeros((P, P), np.float32)], 1).astype(bf),
         np.concatenate([np.ones((P, P), np.float32), tril], 1).astype(bf)]
    xb = np.asarray(x, np.float32).astype(bf)
    wqb = np.asarray(Wq, np.float32).astype(bf)
    wkb = np.asarray(Wk, np.float32).astype(bf)
    wvb = np.asarray(Wv, np.float32).astype(bf)
    in_maps = []
    for c in range(NCORES):
        b, pair = (c // 2) % n_batch, c % 2
        qrows = np.concatenate(
            [xb[b, (2 * j + pair) * P:(2 * j + pair + 1) * P, :]
             for j in range(n_qt)], 0)
        in_maps.append({
            "xq": np.ascontiguousarray(qrows),
            "xkv": np.ascontiguousarray(xb[b]),
            "wq": wqb, "wk": wkb, "wv": wvb,
            "maskt": m[pair], "ident": eye,
        })
    return in_maps


def gather_output(results, T_kv=T, n_qt=None, n_batch=B):
    if n_qt is None:
        n_qt = T_kv // 256
    out = np.empty((n_batch, T_kv, H), np.float32)
    for c in range(2 * n_batch):
        b, pair = c // 2, c % 2
        r = results[c]["outp"]
        for j in range(n_qt):
            out[b, (2 * j + pair) * P:(2 * j + pair + 1) * P, :] = \
                r[j * P:(j + 1) * P, :]
    return out


_NC_CACHE = {}


def kernel(x, Wq, Wk, Wv):
    from concourse.bass_utils import run_bass_kernel_spmd

    x = np.asarray(x, dtype=np.float32)
    Wq = np.asarray(Wq, dtype=np.float32)
    Wk = np.asarray(Wk, dtype=np.float32)
    Wv = np.asarray(Wv, dtype=np.float32)

    if "nc" not in _NC_CACHE:
        _NC_CACHE["nc"] = build_module()
    nc = _NC_CACHE["nc"]

    in_maps = host_inputs(x, Wq, Wk, Wv)
    res = run_bass_kernel_spmd(nc, in_maps, core_ids=list(range(NCORES)))
    return gather_output(res.results)
